# revision 16
# baseline (speedup 1.0000x reference)
"""GAT layer (multi-head graph attention) on 8 TRN2 NeuronCores — V2.

Structure (vs V1 baseline):
  phase 1 : projection GEMM sharded 8-way by node rows (98 tiles/core,
            per-core xt input slice); rows packed [proj bf16 | s_src f32 |
            s_tgt f32 | pad] and written to a local DRAM shard.
  CC      : one 8-core AllGather assembles the full 100352-row table
            (pair-shared HBM output).  (mode="pair": each core instead
            computes half the table into pair-shared HBM, tiny AllReduce
            as a fence.)
  phase 2 : destination windows, processed in groups of G=2 windows.
            Per group: 4 dma_gather calls (one per 32768-row src bucket),
            the fwd one-hot is generated on device (tloc vs iota compare),
            the rev one-hot streams from host; s_tgt expansion + softmax
            chain run group-wide; agg/den accumulate in PSUM banks per
            window (w-major matmul order).
"""

import numpy as np
import ml_dtypes

BF16 = ml_dtypes.bfloat16
P = 128


def _ceil(a, b):
    return -(-a // b)


class Cfg2:
    def __init__(self, fp8: bool = False):
        self.fp8 = fp8
        self.N = 100000
        self.E = 800000
        self.HID = 512
        self.HEADS = 8
        self.F = self.HID // self.HEADS
        self.ncores = 8
        self.G = 2
        self.leak = 0.01
        self.bucket = 32768
        self.NTC = 98                      # phase-1 tiles per core
        self.shard = self.NTC * P          # 12544 rows per core
        self.NPAD8 = self.ncores * self.shard   # 100352
        self.NT = self.NPAD8 // P          # 784
        self.NB = _ceil(self.NPAD8, self.bucket)  # 4
        self.NW = self.NTC                 # 98 windows per core
        self.NG = self.NW // self.G        # 49 groups
        # quarter split of each core's shard (tile-aligned) for chunked CC;
        # uneven split keeps gather buckets at [32768,32768,32768,2048] rows
        # (same slot padding as a single CC) while letting CC_q pipeline
        # behind phase-1 quarter q.
        self.qtiles = [32, 32, 32, 2]
        self.qrows = [t * P for t in self.qtiles]           # per-core rows
        self.qstart = np.cumsum([0] + self.qrows)[:-1]      # within-shard row
        self.qtstart = np.cumsum([0] + self.qtiles)[:-1]    # within-shard tile
        self.brows = [self.ncores * r for r in self.qrows]  # bucket rows
        self.KP = min(self.HID, P)
        self.KT = self.HID // self.KP
        self.proj_bytes = self.HID * (1 if fp8 else 2)
        row_bytes = self.proj_bytes + 2 * self.HEADS * 4
        self.row_bytes = _ceil(row_bytes, 256) * 256   # 768 fp8 / 1280 bf16
        self.row_bf = self.row_bytes // 2
        self.row_f32 = self.row_bytes // 4
        self.s_src_off = self.proj_bytes // 4          # f32 units
        self.s_tgt_off = self.s_src_off + self.HEADS


class Sched2:
    """Uniform (core-independent) grouped phase-2 schedule."""

    def __init__(self, cfg: Cfg2, counts: np.ndarray):
        # counts: [ncores, NW, NB]
        maxcnt = counts.max(axis=0)
        self.seg = np.where(maxcnt > 0, _ceil(maxcnt, P) * P, 0).astype(np.int64)
        self.TG = []          # tiles per group
        self.calls = []       # per group: (b, slot_off, nslots, idxcol0)
        self.wsel = []        # per group: window-in-group of each tile
        self.tslot = []       # per group: first slot of each tile
        self.tile_base = []   # first global tile index of each group
        idxcol = 0
        tt = 0
        for g in range(cfg.NG):
            ws = [cfg.G * g + i for i in range(cfg.G)]
            gcalls, gwsel, gtslot = [], [], []
            off = 0
            for b in range(cfg.NB):
                ns = int(sum(self.seg[w, b] for w in ws))
                if ns == 0:
                    continue
                gcalls.append((b, off, ns, idxcol))
                for wi, w in enumerate(ws):
                    s = int(self.seg[w, b])
                    for j in range(s // P):
                        gwsel.append(wi)
                        gtslot.append(off + sum(int(self.seg[w2, b])
                                                for w2 in ws[:wi]) + j * P)
                off += ns
                idxcol += ns // 16
            assert off % P == 0
            self.tile_base.append(tt)
            self.TG.append(off // P)
            tt += off // P
            self.calls.append(gcalls)
            self.wsel.append(gwsel)
            self.tslot.append(gtslot)
        self.TGmax = max(self.TG)
        self.TT = tt
        self.idxcols = idxcol


def prep_core2(cfg: Cfg2, sched: Sched2, eidx, ebuck, trg, k):
    """Per-core streams: g1i idx wrap, tlocP, ohrev."""
    mask = (trg // cfg.shard) == k
    esrc = eidx[mask]
    etrg = trg[mask]
    trel = etrg - k * cfg.shard
    win = trel // P
    buck = ebuck[mask]
    order = np.lexsort((esrc, buck, win))  # src-sorted: ascending gather addrs
    esrc, trel, win, buck = (a[order] for a in (esrc, trel, win, buck))

    key = win * cfg.NB + buck
    starts = np.searchsorted(key, np.arange(cfg.NW * cfg.NB), side="left")
    ends = np.searchsorted(key, np.arange(cfg.NW * cfg.NB), side="right")

    g1i = np.zeros((P, sched.idxcols), np.int16)
    tlocP = np.full((P, sched.TT), 200.0, BF16)
    ohrev = np.zeros((P, sched.TT, P), BF16)

    for g in range(cfg.NG):
        ws = [cfg.G * g + i for i in range(cfg.G)]
        tbase = sched.tile_base[g]
        tof = 0  # tile offset within group
        for (b, slot_off, nslots, idxcol0) in sched.calls[g]:
            idx = np.zeros(nslots, np.int16)
            soff = 0
            for w in ws:
                s = int(sched.seg[w, b])
                if s == 0:
                    continue
                lo, hi = int(starts[w * cfg.NB + b]), int(ends[w * cfg.NB + b])
                cnt = hi - lo
                assert cnt <= s
                idx[soff:soff + cnt] = esrc[lo:hi].astype(np.int16)
                # per-tile tloc / rev one-hot
                tl = (trel[lo:hi] - w * P).astype(np.int64)
                for j in range(s // P):
                    t = tbase + tof
                    s0, s1 = j * P, min((j + 1) * P, cnt)
                    if s1 > s0:
                        rows = np.arange(s0, s1) - s0
                        cols = tl[s0:s1]
                        tlocP[rows, t] = cols.astype(BF16)
                        ohrev[cols, t, rows] = BF16(1.0)
                    tof += 1
                soff += s
            blk = idx.reshape(nslots // 16, 16).T
            g1i[:, idxcol0:idxcol0 + nslots // 16] = np.tile(blk, (8, 1))
    return g1i, tlocP, ohrev


def pack_xt2(cfg: Cfg2, X: np.ndarray):
    """X [N, HID] f32 -> per-core bf16 [KP, NTC, KT, P] slices."""
    Xp = np.zeros((cfg.NPAD8, cfg.HID), np.float32)
    Xp[: cfg.N] = X
    Xb = Xp.astype(BF16)
    v = Xb.reshape(cfg.NT, P, cfg.KT, cfg.KP).transpose(3, 0, 2, 1)  # [KP,NT,KT,P]
    return [np.ascontiguousarray(v[:, k * cfg.NTC:(k + 1) * cfg.NTC])
            for k in range(cfg.ncores)]


def pack_w2(cfg: Cfg2, W, a_src, a_tgt):
    WT = W.T.astype(np.float32)
    wa_s = (W.reshape(cfg.HEADS, cfg.F, cfg.HID)
            * np.asarray(a_src, np.float32).reshape(cfg.HEADS, cfg.F, 1)).sum(1)
    wa_t = (W.reshape(cfg.HEADS, cfg.F, cfg.HID)
            * np.asarray(a_tgt, np.float32).reshape(cfg.HEADS, cfg.F, 1)).sum(1)
    WA = np.concatenate([wa_s.T, wa_t.T], axis=1)
    wt = np.ascontiguousarray(
        WT.astype(BF16).reshape(cfg.KT, cfg.KP, cfg.HID).transpose(1, 0, 2))
    wa = np.ascontiguousarray(
        WA.astype(BF16).reshape(cfg.KT, cfg.KP, 2 * cfg.HEADS).transpose(1, 0, 2))
    return wt, wa


def _bcast_last(ap, n):
    import concourse.bass as bass
    lst = [list(x) for x in ap.ap] + [[0, n]]
    return bass.AP(ap.tensor, ap.offset, lst)


def _bcast_mid(ap, n):
    """Insert a 0-stride dim of size n before the last dim of an AP."""
    import concourse.bass as bass
    lst = [list(x) for x in ap.ap]
    lst = lst[:-1] + [[0, n]] + lst[-1:]
    return bass.AP(ap.tensor, ap.offset, lst)


def build_nc2(cfg: Cfg2, sched: Sched2, mode: str = "allgather",
              repeat: int = 1, cc: str = "on", skip: str = ""):
    skips = set(s for s in skip.split(",") if s)
    import concourse.bacc as bacc
    import concourse.bass as bass
    import concourse.mybir as mybir
    from concourse.tile import TileContext

    dt = mybir.dt
    H, HID, KT, KP, G = cfg.HEADS, cfg.HID, cfg.KT, cfg.KP, cfg.G

    nc = bacc.Bacc("TRN2", target_bir_lowering=False, num_devices=cfg.ncores)

    xt = nc.dram_tensor("xt", [KP, cfg.NTC, KT, P], dt.bfloat16,
                        kind="ExternalInput")
    wt = nc.dram_tensor("wt", [KP, KT, HID], dt.bfloat16, kind="ExternalInput")
    wa = nc.dram_tensor("wa", [KP, KT, 2 * H], dt.bfloat16, kind="ExternalInput")
    g1i = nc.dram_tensor("g1i", [P, sched.idxcols], dt.int16, kind="ExternalInput")
    tlocd = nc.dram_tensor("tlocd", [P, sched.TT], dt.bfloat16,
                           kind="ExternalInput")
    ohrevd = nc.dram_tensor("ohrevd", [P, sched.TT, P], dt.bfloat16,
                            kind="ExternalInput")
    iotad = nc.dram_tensor("iotad", [P, P], dt.bfloat16, kind="ExternalInput")
    avec = nc.dram_tensor("avec", [P, 1], dt.float32, kind="ExternalInput")
    out = nc.dram_tensor("out", [cfg.NW * P, HID], dt.float32,
                         kind="ExternalOutput")

    with TileContext(nc) as tc:
        with tc.tile_pool(name="const", bufs=1) as cpool, \
             tc.tile_pool(name="dram", bufs=1, space="DRAM") as dpool:
            tshard = dpool.tile([cfg.shard, cfg.row_bf], dt.bfloat16)
            if mode == "allgather4":
                tableQ = [dpool.tile([cfg.brows[q], cfg.row_bf], dt.bfloat16,
                                     addr_space="Shared", name=f"tableQ{q}")
                          for q in range(4)]
            else:
                table = dpool.tile([cfg.NPAD8, cfg.row_bf], dt.bfloat16,
                                   addr_space="Shared")
            wt_sb = cpool.tile([KP, KT, HID], dt.bfloat16)
            nc.sync.dma_start(out=wt_sb[:], in_=wt[:, :, :])
            wa_sb = cpool.tile([KP, KT, 2 * H], dt.bfloat16)
            nc.sync.dma_start(out=wa_sb[:], in_=wa[:, :, :])
            a_sb = cpool.tile([P, 1], dt.float32)
            nc.sync.dma_start(out=a_sb[:], in_=avec[:, :])
            g1i_sb = cpool.tile([P, sched.idxcols], dt.int16)
            nc.sync.dma_start(out=g1i_sb[:], in_=g1i[:, :])
            tloc_sb = cpool.tile([P, sched.TT], dt.bfloat16)
            nc.sync.dma_start(out=tloc_sb[:], in_=tlocd[:, :])
            iota_sb = cpool.tile([P, P], dt.bfloat16)
            nc.sync.dma_start(out=iota_sb[:], in_=iotad[:, :])

            # ---------------- phase 1: projection table shard ----------------
            def emit_phase1(rep):
                with tc.tile_pool(name=f"p1_{rep}", bufs=3) as xpool, \
                     tc.tile_pool(name=f"p1ps_{rep}", bufs=2, space="PSUM") as psp, \
                     tc.tile_pool(name=f"p1st_{rep}", bufs=3) as stpool:
                    if "p1" in skips:
                        stg = stpool.tile([P, 2, cfg.row_bf], dt.bfloat16,
                                          tag="stg")
                        nc.vector.memset(stg[:], 0.0)
                        nc.sync.dma_start(
                            out=tshard[0:2 * P, :].rearrange(
                                "(two p) r -> p two r", p=P),
                            in_=stg[:])
                        return
                    for j0 in range(0, cfg.NTC, 2):
                        xtile = xpool.tile([KP, 2, KT, P], dt.bfloat16, tag="x")
                        nc.scalar.dma_start(out=xtile[:],
                                            in_=xt[:, j0:j0 + 2, :, :])
                        stg = stpool.tile([P, 2, cfg.row_bf], dt.bfloat16,
                                          tag="stg")
                        stg32 = stg.bitcast(dt.float32)
                        for u in range(2):
                            ps1 = psp.tile([P, HID], dt.float32, tag="ps1")
                            ps2 = psp.tile([P, 2 * H], dt.float32, tag="ps2")
                            for ki in range(KT):
                                nc.tensor.matmul(ps1[:], xtile[:, u, ki, :],
                                                 wt_sb[:, ki, :],
                                                 start=(ki == 0),
                                                 stop=(ki == KT - 1))
                            for ki in range(KT):
                                nc.tensor.matmul(ps2[:], xtile[:, u, ki, :],
                                                 wa_sb[:, ki, :],
                                                 start=(ki == 0),
                                                 stop=(ki == KT - 1))
                            if cfg.fp8:
                                stg8 = stg.bitcast(dt.float8e4)
                                nc.scalar.copy(out=stg8[:, u, 0:HID], in_=ps1[:])
                            else:
                                nc.scalar.copy(out=stg[:, u, 0:HID], in_=ps1[:])
                            nc.vector.tensor_copy(
                                out=stg32[:, u,
                                          cfg.s_src_off:cfg.s_src_off + 2 * H],
                                in_=ps2[:])
                            nc.vector.memset(
                                stg32[:, u, cfg.s_tgt_off + H:cfg.row_f32], 0.0)
                        nc.sync.dma_start(
                            out=tshard[j0 * P:(j0 + 2) * P, :].rearrange(
                                "(two p) r -> p two r", p=P),
                            in_=stg[:])

            # ---------------- phase 1.5: resident s_tgt (hi/lo) --------------
            def emit_phase15():
                ts32 = tshard.bitcast(dt.float32)
                s_ap = ts32[:, cfg.s_tgt_off:cfg.s_tgt_off + H]
                s_ap = s_ap.rearrange("(w p) h -> p w h", p=P)
                s_all = cpool.tile([P, cfg.NW, H], dt.float32)
                nc.sync.dma_start(out=s_all[:], in_=s_ap)
                s_hilo = cpool.tile([P, cfg.NW, 2, H], dt.bfloat16)
                s_hi32 = cpool.tile([P, cfg.NW, H], dt.float32)
                nc.vector.tensor_copy(out=s_hilo[:, :, 0, :], in_=s_all[:])
                nc.vector.tensor_copy(out=s_hi32[:], in_=s_hilo[:, :, 0, :])
                nc.vector.tensor_tensor(out=s_hilo[:, :, 1, :], in0=s_all[:],
                                        in1=s_hi32[:],
                                        op=mybir.AluOpType.subtract)
                return s_hilo

            # ---------------- collective: assemble full table ----------------
            def emit_cc():
                groups = [list(range(cfg.ncores))]
                if mode == "allgather4":
                    for q in range(4):
                        if cc == "off":
                            nc.gpsimd.dma_start(out=tableQ[q][0:P, :],
                                                in_=tshard[0:P, :])
                            continue
                        lo = int(cfg.qstart[q])
                        nc.gpsimd.collective_compute(
                            "AllGather", mybir.AluOpType.bypass,
                            replica_groups=groups,
                            ins=[tshard[lo:lo + cfg.qrows[q], :].opt()],
                            outs=[tableQ[q].opt()],
                        )
                    return
                if cc == "off":
                    # timing-only ablation: satisfy write-before-read
                    nc.gpsimd.dma_start(out=table[0:P, :], in_=tshard[0:P, :])
                    return
                nc.gpsimd.collective_compute(
                    "AllGather",
                    mybir.AluOpType.bypass,
                    replica_groups=groups,
                    ins=[tshard.opt()],
                    outs=[table.opt()],
                )

            # ---------------- phase 2: window groups --------------------------
            def emit_phase2(rep, s_hilo):
                with tc.tile_pool(name=f"p2_{rep}", bufs=2) as pool, \
                     tc.tile_pool(name=f"p2ps_{rep}", bufs=2, space="PSUM") as pps:
                    for g in range(cfg.NG):
                        emit_group(pool, pps, s_hilo, g)

            def emit_group(pool, pps, s_hilo, g):
                TG = sched.TG[g]
                wsel = sched.wsel[g]
                tbase = sched.tile_base[g]
                g1t = pool.tile([P, sched.TGmax, cfg.row_bf], dt.bfloat16,
                                tag="g1t", bufs=3)
                if "gather" in skips:
                    nc.vector.memset(g1t[:, 0:1, 0:1], 0.0)
                for (b, slot_off, nslots, idxcol0) in \
                        ([] if "gather" in skips else sched.calls[g]):
                    if mode == "allgather4":
                        src_ap = tableQ[b][:, :]
                    else:
                        rows = min(cfg.NPAD8,
                                   (b + 1) * cfg.bucket) - b * cfg.bucket
                        src_ap = table[b * cfg.bucket:b * cfg.bucket + rows, :]
                    nc.gpsimd.dma_gather(
                        g1t[:, slot_off // P:(slot_off + nslots) // P, :],
                        src_ap,
                        g1i_sb[:, idxcol0:idxcol0 + nslots // 16],
                        nslots, nslots, cfg.row_bf)
                ohrev_t = pool.tile([P, sched.TGmax, P], dt.bfloat16,
                                    tag="ohrev", bufs=3)
                if "ohrev" in skips:
                    nc.vector.memset(ohrev_t[:, 0:1, 0:1], 0.0)
                else:
                    nc.scalar.dma_start(out=ohrev_t[:, :TG, :],
                                        in_=ohrevd[:, tbase:tbase + TG, :])
                fwd = pool.tile([P, sched.TGmax, P], dt.bfloat16, tag="fwd",
                                bufs=3)
                if "fwd" in skips:
                    nc.vector.memset(fwd[:, 0:1, 0:1], 0.0)
                else:
                    nc.vector.tensor_tensor(
                        out=fwd[:, :TG, :],
                        in0=_bcast_last(tloc_sb[:, tbase:tbase + TG], P),
                        in1=_bcast_mid(iota_sb[:, :], TG),
                        op=mybir.AluOpType.is_equal)

                # s_tgt expansion into slot space (per tile); hi+lo summed in PSUM
                stgt = pps.tile([P, sched.TGmax, H], dt.float32, tag="stgt")
                if "mm" in skips:
                    nc.vector.memset(stgt[:, 0:1, 0:1], 0.0)
                for t in range(0 if "mm" in skips else TG):
                    w = G * g + wsel[t]
                    nc.tensor.matmul(stgt[:, t, :], ohrev_t[:, t, :],
                                     s_hilo[:, w, 0, :], start=True, stop=False)
                    nc.tensor.matmul(stgt[:, t, :], ohrev_t[:, t, :],
                                     s_hilo[:, w, 1, :], start=False, stop=True)

                if "vec" in skips:
                    res = pool.tile([P, G, HID], dt.float32, tag="res")
                    nc.vector.memset(res[:, 0:1, 0:1], 0.0)
                    o_ap = out[G * g * P:(G * g + G) * P, :]
                    o_ap = o_ap.rearrange("(w p) h -> p w h", p=P)
                    nc.sync.dma_start(out=o_ap, in_=res[:])
                    return
                # group-wide softmax chain
                g1t32 = g1t.bitcast(dt.float32)
                s_sum = pool.tile([P, sched.TGmax, H], dt.float32, tag="s_sum")
                s_act = pool.tile([P, sched.TGmax, H], dt.float32, tag="s_act")
                nc.vector.tensor_tensor(
                    out=s_sum[:, :TG, :], in0=stgt[:, :TG, :],
                    in1=g1t32[:, :TG, cfg.s_src_off:cfg.s_src_off + H],
                    op=mybir.AluOpType.add)
                nc.vector.scalar_tensor_tensor(
                    out=s_act[:, :TG, :], in0=s_sum[:, :TG, :], scalar=cfg.leak,
                    in1=s_sum[:, :TG, :], op0=mybir.AluOpType.mult,
                    op1=mybir.AluOpType.max)
                exp_t = pool.tile([P, sched.TGmax, H], dt.bfloat16, tag="exp_t")
                nc.scalar.activation(out=exp_t[:, :TG, :], in_=s_act[:, :TG, :],
                                     func=mybir.ActivationFunctionType.Exp)
                w_t = pool.tile([P, sched.TGmax, HID], dt.bfloat16, tag="w_t")
                if cfg.fp8:
                    projv = g1t.bitcast(dt.float8e4)[:, :TG, 0:HID]
                else:
                    projv = g1t[:, :TG, 0:HID]
                proj4 = projv.rearrange("p t (h f) -> p t h f", h=H)
                exp4 = _bcast_last(exp_t[:, :TG, :], cfg.F)
                out4 = w_t[:, :TG, :].rearrange("p t (h f) -> p t h f", h=H)
                nc.vector.tensor_tensor(out=out4, in0=proj4, in1=exp4,
                                        op=mybir.AluOpType.mult)

                # aggregation, w-major order
                agg = pps.tile([P, G, HID], dt.float32, tag="agg")
                den = pps.tile([P, G, H], dt.float32, tag="den")
                if "mm" in skips:
                    nc.vector.memset(agg[:, 0:1, 0:1], 0.0)
                    nc.vector.memset(den[:, 0:1, 0:1], 0.0)
                for wi in range(0 if "mm" in skips else G):
                    ts = [t for t in range(TG) if wsel[t] == wi]
                    for i, t in enumerate(ts):
                        st, sp = (i == 0), (i == len(ts) - 1)
                        nc.tensor.matmul(agg[:, wi, :], fwd[:, t, :],
                                         w_t[:, t, :], start=st, stop=sp)
                        nc.tensor.matmul(den[:, wi, :], fwd[:, t, :],
                                         exp_t[:, t, :], start=st, stop=sp)

                # flush both windows
                den_sb = pool.tile([P, G, H, 1], dt.float32, tag="den_sb")
                recip = pool.tile([P, G, H, 1], dt.float32, tag="recip")
                nc.vector.tensor_scalar_add(out=den_sb[:, :, :, 0], in0=den[:],
                                            scalar1=1e-16)
                nc.vector.reciprocal(out=recip[:], in_=den_sb[:])
                z = pool.tile([P, G, HID], dt.float32, tag="z")
                agg4 = agg[:].rearrange("p w (h f) -> p w h f", h=H)
                z4 = z[:].rearrange("p w (h f) -> p w h f", h=H)
                nc.vector.tensor_tensor(out=z4, in0=agg4,
                                        in1=_bcast_last(recip[:, :, :, 0], cfg.F),
                                        op=mybir.AluOpType.mult)
                res = pool.tile([P, G, HID], dt.float32, tag="res")
                nc.vector.scalar_tensor_tensor(
                    out=res[:], in0=z[:], scalar=a_sb[:, 0:1], in1=z[:],
                    op0=mybir.AluOpType.mult, op1=mybir.AluOpType.max)
                o_ap = out[G * g * P:(G * g + G) * P, :]
                o_ap = o_ap.rearrange("(w p) h -> p w h", p=P)
                nc.sync.dma_start(out=o_ap, in_=res[:])

            for rep in range(repeat):
                emit_phase1(rep)
                s_hilo = emit_phase15()
                if rep == 0:
                    emit_cc()
                emit_phase2(rep, s_hilo)
                if rep < repeat - 1:
                    tc.strict_bb_all_engine_barrier()

    nc.compile()
    return nc


def prepare2(cfg: Cfg2, inputs, mode: str = "allgather"):
    X = np.asarray(inputs["in_nodes_features"], np.float32)
    ei = np.asarray(inputs["edge_index"], np.int64)
    W = np.asarray(inputs["W"], np.float32)
    b_lin = np.asarray(inputs["b_lin"], np.float32)
    a_src = np.asarray(inputs["a_src"], np.float32)
    a_tgt = np.asarray(inputs["a_tgt"], np.float32)
    bias = np.asarray(inputs["bias"], np.float32)
    prelu_a = float(np.asarray(inputs["prelu_a"], np.float32))

    assert np.all(b_lin == 0) and np.all(bias == 0), "nonzero bias unsupported"
    assert 0.0 <= prelu_a <= 1.0, "prelu_a outside [0,1] unsupported"

    src, trg = ei[0], ei[1]
    core_of = trg // cfg.shard
    win_of = (trg % cfg.shard) // P
    if mode == "allgather4":
        sk = src // cfg.shard
        sj = src % cfg.shard
        qs = np.asarray(cfg.qstart, np.int64)
        qr = np.asarray(cfg.qrows, np.int64)
        buck_of = np.searchsorted(qs, sj, side="right") - 1
        eidx = sk * qr[buck_of] + (sj - qs[buck_of])
    else:
        buck_of = src // cfg.bucket
        eidx = src - buck_of * cfg.bucket
    counts = np.zeros((cfg.ncores, cfg.NW, cfg.NB), np.int64)
    for k in range(cfg.ncores):
        m = core_of == k
        counts[k] = np.bincount(
            win_of[m] * cfg.NB + buck_of[m],
            minlength=cfg.NW * cfg.NB).reshape(cfg.NW, cfg.NB)
    sched = Sched2(cfg, counts)

    xts = pack_xt2(cfg, X)
    wtp, wap = pack_w2(cfg, W, a_src, a_tgt)
    av = np.full((P, 1), prelu_a, np.float32)
    iota = np.broadcast_to(np.arange(P, dtype=np.float32), (P, P)).astype(BF16)
    iota = np.ascontiguousarray(iota)

    in_maps = []
    for k in range(cfg.ncores):
        g1i_k, tloc_k, ohrev_k = prep_core2(cfg, sched, eidx, buck_of, trg, k)
        in_maps.append({
            "xt": xts[k], "wt": wtp, "wa": wap, "g1i": g1i_k,
            "tlocd": tloc_k, "ohrevd": ohrev_k, "iotad": iota, "avec": av,
        })

    def assemble(core_outs):
        full = np.concatenate(
            [np.asarray(o["out"], np.float32) for o in core_outs], axis=0)
        return full[: cfg.N]

    return sched, in_maps, assemble


_BUILT2 = {}


MODE = "allgather4"


def kernel(**inputs):
    from concourse.bass_utils import run_bass_kernel_spmd

    cfg = Cfg2()
    sched, in_maps, assemble = prepare2(cfg, inputs, mode=MODE)
    key = (tuple(sched.TG), sched.idxcols)
    if key not in _BUILT2:
        _BUILT2[key] = build_nc2(cfg, sched, mode=MODE)
    nc = _BUILT2[key]
    res = run_bass_kernel_spmd(nc, in_maps, core_ids=list(range(cfg.ncores)))
    return assemble(res.results)


# --- back-compat aliases so the existing test.py harness keeps working ---
Cfg = Cfg2


def prepare(cfg, inputs):
    return prepare2(cfg, inputs, mode=MODE)


def build_nc(cfg, sched, phases="full", skip="", repeat=1):
    return build_nc2(cfg, sched, mode=MODE, repeat=repeat, skip=skip)


# revision 17
# speedup vs baseline: 1.1992x; 1.1992x over previous
"""GAT layer (multi-head graph attention) on 8 TRN2 NeuronCores — V2.

Structure (vs V1 baseline):
  phase 1 : projection GEMM sharded 8-way by node rows (98 tiles/core,
            per-core xt input slice); rows packed [proj bf16 | s_src f32 |
            s_tgt f32 | pad] and written to a local DRAM shard.
  CC      : one 8-core AllGather assembles the full 100352-row table
            (pair-shared HBM output).  (mode="pair": each core instead
            computes half the table into pair-shared HBM, tiny AllReduce
            as a fence.)
  phase 2 : destination windows, processed in groups of G=2 windows.
            Per group: 4 dma_gather calls (one per 32768-row src bucket),
            the fwd one-hot is generated on device (tloc vs iota compare),
            the rev one-hot streams from host; s_tgt expansion + softmax
            chain run group-wide; agg/den accumulate in PSUM banks per
            window (w-major matmul order).
"""

import numpy as np
import ml_dtypes

BF16 = ml_dtypes.bfloat16
P = 128


def _ceil(a, b):
    return -(-a // b)


class Cfg2:
    def __init__(self, fp8: bool = False):
        self.fp8 = fp8
        self.N = 100000
        self.E = 800000
        self.HID = 512
        self.HEADS = 8
        self.F = self.HID // self.HEADS
        self.ncores = 8
        self.G = 2
        self.leak = 0.01
        self.bucket = 32768
        self.NTC = 98                      # phase-1 tiles per core
        self.shard = self.NTC * P          # 12544 rows per core
        self.NPAD8 = self.ncores * self.shard   # 100352
        self.NT = self.NPAD8 // P          # 784
        self.NB = _ceil(self.NPAD8, self.bucket)  # 4
        self.NW = self.NTC                 # 98 windows per core
        self.NG = self.NW // self.G        # 49 groups
        # quarter split of each core's shard (tile-aligned) for chunked CC;
        # uneven split keeps gather buckets at [32768,32768,32768,2048] rows
        # (same slot padding as a single CC) while letting CC_q pipeline
        # behind phase-1 quarter q.
        self.qtiles = [32, 32, 32, 2]
        self.qrows = [t * P for t in self.qtiles]           # per-core rows
        self.qstart = np.cumsum([0] + self.qrows)[:-1]      # within-shard row
        self.qtstart = np.cumsum([0] + self.qtiles)[:-1]    # within-shard tile
        self.brows = [self.ncores * r for r in self.qrows]  # bucket rows
        self.KP = min(self.HID, P)
        self.KT = self.HID // self.KP
        self.proj_bytes = self.HID * (1 if fp8 else 2)
        row_bytes = self.proj_bytes + 2 * self.HEADS * 4
        self.row_bytes = _ceil(row_bytes, 256) * 256   # 768 fp8 / 1280 bf16
        self.row_bf = self.row_bytes // 2
        self.row_f32 = self.row_bytes // 4
        self.s_src_off = self.proj_bytes // 4          # f32 units
        self.s_tgt_off = self.s_src_off + self.HEADS


class Sched2:
    """Uniform (core-independent) grouped phase-2 schedule."""

    def __init__(self, cfg: Cfg2, counts: np.ndarray):
        # counts: [ncores, NW, NB]
        maxcnt = counts.max(axis=0)
        self.seg = np.where(maxcnt > 0, _ceil(maxcnt, P) * P, 0).astype(np.int64)
        self.TG = []          # tiles per group
        self.calls = []       # per group: (b, slot_off, nslots, idxcol0)
        self.wsel = []        # per group: window-in-group of each tile
        self.tslot = []       # per group: first slot of each tile
        self.tile_base = []   # first global tile index of each group
        idxcol = 0
        tt = 0
        for g in range(cfg.NG):
            ws = [cfg.G * g + i for i in range(cfg.G)]
            gcalls, gwsel, gtslot = [], [], []
            off = 0
            for b in range(cfg.NB):
                ns = int(sum(self.seg[w, b] for w in ws))
                if ns == 0:
                    continue
                gcalls.append((b, off, ns, idxcol))
                for wi, w in enumerate(ws):
                    s = int(self.seg[w, b])
                    for j in range(s // P):
                        gwsel.append(wi)
                        gtslot.append(off + sum(int(self.seg[w2, b])
                                                for w2 in ws[:wi]) + j * P)
                off += ns
                idxcol += ns // 16
            assert off % P == 0
            self.tile_base.append(tt)
            self.TG.append(off // P)
            tt += off // P
            self.calls.append(gcalls)
            self.wsel.append(gwsel)
            self.tslot.append(gtslot)
        self.TGmax = max(self.TG)
        self.TT = tt
        self.idxcols = idxcol


def prep_core2(cfg: Cfg2, sched: Sched2, eidx, ebuck, trg, k):
    """Per-core streams: g1i idx wrap, tlocP, ohrev."""
    mask = (trg // cfg.shard) == k
    esrc = eidx[mask]
    etrg = trg[mask]
    trel = etrg - k * cfg.shard
    win = trel // P
    buck = ebuck[mask]
    order = np.lexsort((esrc, buck, win))  # src-sorted: ascending gather addrs
    esrc, trel, win, buck = (a[order] for a in (esrc, trel, win, buck))

    key = win * cfg.NB + buck
    starts = np.searchsorted(key, np.arange(cfg.NW * cfg.NB), side="left")
    ends = np.searchsorted(key, np.arange(cfg.NW * cfg.NB), side="right")

    g1i = np.zeros((P, sched.idxcols), np.int16)
    tlocP = np.full((P, sched.TT), 200.0, BF16)
    ohrev = np.zeros((P, sched.TT, P), BF16)

    for g in range(cfg.NG):
        ws = [cfg.G * g + i for i in range(cfg.G)]
        tbase = sched.tile_base[g]
        tof = 0  # tile offset within group
        for (b, slot_off, nslots, idxcol0) in sched.calls[g]:
            idx = np.zeros(nslots, np.int16)
            soff = 0
            for w in ws:
                s = int(sched.seg[w, b])
                if s == 0:
                    continue
                lo, hi = int(starts[w * cfg.NB + b]), int(ends[w * cfg.NB + b])
                cnt = hi - lo
                assert cnt <= s
                idx[soff:soff + cnt] = esrc[lo:hi].astype(np.int16)
                # per-tile tloc / rev one-hot
                tl = (trel[lo:hi] - w * P).astype(np.int64)
                for j in range(s // P):
                    t = tbase + tof
                    s0, s1 = j * P, min((j + 1) * P, cnt)
                    if s1 > s0:
                        rows = np.arange(s0, s1) - s0
                        cols = tl[s0:s1]
                        tlocP[rows, t] = cols.astype(BF16)
                        ohrev[cols, t, rows] = BF16(1.0)
                    tof += 1
                soff += s
            blk = idx.reshape(nslots // 16, 16).T
            g1i[:, idxcol0:idxcol0 + nslots // 16] = np.tile(blk, (8, 1))
    return g1i, tlocP, ohrev


def pack_xt2(cfg: Cfg2, X: np.ndarray):
    """X [N, HID] f32 -> per-core bf16 [KP, NTC, KT, P] slices."""
    Xp = np.zeros((cfg.NPAD8, cfg.HID), np.float32)
    Xp[: cfg.N] = X
    Xb = Xp.astype(BF16)
    v = Xb.reshape(cfg.NT, P, cfg.KT, cfg.KP).transpose(3, 0, 2, 1)  # [KP,NT,KT,P]
    return [np.ascontiguousarray(v[:, k * cfg.NTC:(k + 1) * cfg.NTC])
            for k in range(cfg.ncores)]


def pack_w2(cfg: Cfg2, W, a_src, a_tgt):
    WT = W.T.astype(np.float32)
    wa_s = (W.reshape(cfg.HEADS, cfg.F, cfg.HID)
            * np.asarray(a_src, np.float32).reshape(cfg.HEADS, cfg.F, 1)).sum(1)
    wa_t = (W.reshape(cfg.HEADS, cfg.F, cfg.HID)
            * np.asarray(a_tgt, np.float32).reshape(cfg.HEADS, cfg.F, 1)).sum(1)
    WA = np.concatenate([wa_s.T, wa_t.T], axis=1)
    wt = np.ascontiguousarray(
        WT.astype(BF16).reshape(cfg.KT, cfg.KP, cfg.HID).transpose(1, 0, 2))
    wa = np.ascontiguousarray(
        WA.astype(BF16).reshape(cfg.KT, cfg.KP, 2 * cfg.HEADS).transpose(1, 0, 2))
    return wt, wa


def _bcast_last(ap, n):
    import concourse.bass as bass
    lst = [list(x) for x in ap.ap] + [[0, n]]
    return bass.AP(ap.tensor, ap.offset, lst)


def _bcast_mid(ap, n):
    """Insert a 0-stride dim of size n before the last dim of an AP."""
    import concourse.bass as bass
    lst = [list(x) for x in ap.ap]
    lst = lst[:-1] + [[0, n]] + lst[-1:]
    return bass.AP(ap.tensor, ap.offset, lst)


def build_nc2(cfg: Cfg2, sched: Sched2, mode: str = "allgather",
              repeat: int = 1, cc: str = "on", skip: str = ""):
    skips = set(s for s in skip.split(",") if s)
    import concourse.bacc as bacc
    import concourse.bass as bass
    import concourse.mybir as mybir
    from concourse.tile import TileContext

    dt = mybir.dt
    H, HID, KT, KP, G = cfg.HEADS, cfg.HID, cfg.KT, cfg.KP, cfg.G

    nc = bacc.Bacc("TRN2", target_bir_lowering=False, num_devices=cfg.ncores)

    xt = nc.dram_tensor("xt", [KP, cfg.NTC, KT, P], dt.bfloat16,
                        kind="ExternalInput")
    wt = nc.dram_tensor("wt", [KP, KT, HID], dt.bfloat16, kind="ExternalInput")
    wa = nc.dram_tensor("wa", [KP, KT, 2 * H], dt.bfloat16, kind="ExternalInput")
    g1i = nc.dram_tensor("g1i", [P, sched.idxcols], dt.int16, kind="ExternalInput")
    tlocd = nc.dram_tensor("tlocd", [P, sched.TT], dt.bfloat16,
                           kind="ExternalInput")
    ohrevd = nc.dram_tensor("ohrevd", [P, sched.TT, P], dt.bfloat16,
                            kind="ExternalInput")
    iotad = nc.dram_tensor("iotad", [P, P], dt.bfloat16, kind="ExternalInput")
    avec = nc.dram_tensor("avec", [P, 1], dt.float32, kind="ExternalInput")
    out = nc.dram_tensor("out", [cfg.NW * P, HID], dt.float32,
                         kind="ExternalOutput")

    with TileContext(nc) as tc:
        with tc.tile_pool(name="const", bufs=1) as cpool, \
             tc.tile_pool(name="dram", bufs=1, space="DRAM") as dpool:
            tshard = dpool.tile([cfg.shard, cfg.row_bf], dt.bfloat16)
            if mode == "allgather4":
                tableQ = [dpool.tile([cfg.brows[q], cfg.row_bf], dt.bfloat16,
                                     addr_space="Shared", name=f"tableQ{q}")
                          for q in range(4)]
            else:
                table = dpool.tile([cfg.NPAD8, cfg.row_bf], dt.bfloat16,
                                   addr_space="Shared")
            wt_sb = cpool.tile([KP, KT, HID], dt.bfloat16)
            nc.sync.dma_start(out=wt_sb[:], in_=wt[:, :, :])
            wa_sb = cpool.tile([KP, KT, 2 * H], dt.bfloat16)
            nc.sync.dma_start(out=wa_sb[:], in_=wa[:, :, :])
            a_sb = cpool.tile([P, 1], dt.float32)
            nc.sync.dma_start(out=a_sb[:], in_=avec[:, :])
            g1i_sb = cpool.tile([P, sched.idxcols], dt.int16)
            nc.sync.dma_start(out=g1i_sb[:], in_=g1i[:, :])
            tloc_sb = cpool.tile([P, sched.TT], dt.bfloat16)
            nc.sync.dma_start(out=tloc_sb[:], in_=tlocd[:, :])
            iota_sb = cpool.tile([P, P], dt.bfloat16)
            nc.sync.dma_start(out=iota_sb[:], in_=iotad[:, :])

            # ---------------- phase 1: projection table shard ----------------
            def emit_phase1(rep):
                with tc.tile_pool(name=f"p1_{rep}", bufs=3) as xpool, \
                     tc.tile_pool(name=f"p1ps_{rep}", bufs=2, space="PSUM") as psp, \
                     tc.tile_pool(name=f"p1st_{rep}", bufs=3) as stpool:
                    if "p1" in skips:
                        stg = stpool.tile([P, 2, cfg.row_bf], dt.bfloat16,
                                          tag="stg")
                        nc.vector.memset(stg[:], 0.0)
                        nc.sync.dma_start(
                            out=tshard[0:2 * P, :].rearrange(
                                "(two p) r -> p two r", p=P),
                            in_=stg[:])
                        return
                    for j0 in range(0, cfg.NTC, 2):
                        xtile = xpool.tile([KP, 2, KT, P], dt.bfloat16, tag="x")
                        nc.scalar.dma_start(out=xtile[:],
                                            in_=xt[:, j0:j0 + 2, :, :])
                        stg = stpool.tile([P, 2, cfg.row_bf], dt.bfloat16,
                                          tag="stg")
                        stg32 = stg.bitcast(dt.float32)
                        for u in range(2):
                            ps1 = psp.tile([P, HID], dt.float32, tag="ps1")
                            ps2 = psp.tile([P, 2 * H], dt.float32, tag="ps2")
                            for ki in range(KT):
                                nc.tensor.matmul(ps1[:], xtile[:, u, ki, :],
                                                 wt_sb[:, ki, :],
                                                 start=(ki == 0),
                                                 stop=(ki == KT - 1))
                            for ki in range(KT):
                                nc.tensor.matmul(ps2[:], xtile[:, u, ki, :],
                                                 wa_sb[:, ki, :],
                                                 start=(ki == 0),
                                                 stop=(ki == KT - 1))
                            if cfg.fp8:
                                stg8 = stg.bitcast(dt.float8e4)
                                nc.scalar.copy(out=stg8[:, u, 0:HID], in_=ps1[:])
                            else:
                                nc.scalar.copy(out=stg[:, u, 0:HID], in_=ps1[:])
                            nc.vector.tensor_copy(
                                out=stg32[:, u,
                                          cfg.s_src_off:cfg.s_src_off + 2 * H],
                                in_=ps2[:])
                            nc.vector.memset(
                                stg32[:, u, cfg.s_tgt_off + H:cfg.row_f32], 0.0)
                        nc.sync.dma_start(
                            out=tshard[j0 * P:(j0 + 2) * P, :].rearrange(
                                "(two p) r -> p two r", p=P),
                            in_=stg[:])

            # ---------------- phase 1.5: resident s_tgt (hi/lo) --------------
            def emit_phase15():
                ts32 = tshard.bitcast(dt.float32)
                s_ap = ts32[:, cfg.s_tgt_off:cfg.s_tgt_off + H]
                s_ap = s_ap.rearrange("(w p) h -> p w h", p=P)
                s_all = cpool.tile([P, cfg.NW, H], dt.float32)
                nc.sync.dma_start(out=s_all[:], in_=s_ap)
                s_hilo = cpool.tile([P, cfg.NW, 2, H], dt.bfloat16)
                s_hi32 = cpool.tile([P, cfg.NW, H], dt.float32)
                nc.vector.tensor_copy(out=s_hilo[:, :, 0, :], in_=s_all[:])
                nc.vector.tensor_copy(out=s_hi32[:], in_=s_hilo[:, :, 0, :])
                nc.vector.tensor_tensor(out=s_hilo[:, :, 1, :], in0=s_all[:],
                                        in1=s_hi32[:],
                                        op=mybir.AluOpType.subtract)
                return s_hilo

            # ---------------- collective: assemble full table ----------------
            def emit_cc():
                groups = [list(range(cfg.ncores))]
                if mode == "allgather4":
                    for q in range(4):
                        if cc == "off":
                            nc.gpsimd.dma_start(out=tableQ[q][0:P, :],
                                                in_=tshard[0:P, :])
                            continue
                        lo = int(cfg.qstart[q])
                        nc.gpsimd.collective_compute(
                            "AllGather", mybir.AluOpType.bypass,
                            replica_groups=groups,
                            ins=[tshard[lo:lo + cfg.qrows[q], :].opt()],
                            outs=[tableQ[q].opt()],
                        )
                    return
                if cc == "off":
                    # timing-only ablation: satisfy write-before-read
                    nc.gpsimd.dma_start(out=table[0:P, :], in_=tshard[0:P, :])
                    return
                nc.gpsimd.collective_compute(
                    "AllGather",
                    mybir.AluOpType.bypass,
                    replica_groups=groups,
                    ins=[tshard.opt()],
                    outs=[table.opt()],
                )

            # ---------------- phase 2: window groups --------------------------
            def emit_phase2(rep, s_hilo):
                with tc.tile_pool(name=f"p2_{rep}", bufs=2) as pool, \
                     tc.tile_pool(name=f"p2ps_{rep}", bufs=2, space="PSUM") as pps:
                    for g in range(cfg.NG):
                        emit_group(pool, pps, s_hilo, g)

            def emit_group(pool, pps, s_hilo, g):
                TG = sched.TG[g]
                wsel = sched.wsel[g]
                tbase = sched.tile_base[g]
                g1t = pool.tile([P, sched.TGmax, cfg.row_bf], dt.bfloat16,
                                tag="g1t", bufs=3)
                if "gather" in skips:
                    nc.vector.memset(g1t[:, 0:1, 0:1], 0.0)
                for (b, slot_off, nslots, idxcol0) in \
                        ([] if "gather" in skips else sched.calls[g]):
                    if mode == "allgather4":
                        src_ap = tableQ[b][:, :]
                    else:
                        rows = min(cfg.NPAD8,
                                   (b + 1) * cfg.bucket) - b * cfg.bucket
                        src_ap = table[b * cfg.bucket:b * cfg.bucket + rows, :]
                    nc.gpsimd.dma_gather(
                        g1t[:, slot_off // P:(slot_off + nslots) // P, :],
                        src_ap,
                        g1i_sb[:, idxcol0:idxcol0 + nslots // 16],
                        nslots, nslots, cfg.row_bf)
                ohrev_t = pool.tile([P, sched.TGmax, P], dt.bfloat16, tag="ohrev")
                if "ohrev" in skips:
                    nc.vector.memset(ohrev_t[:, 0:1, 0:1], 0.0)
                else:
                    nc.scalar.dma_start(out=ohrev_t[:, :TG, :],
                                        in_=ohrevd[:, tbase:tbase + TG, :])
                fwd = pool.tile([P, sched.TGmax, P], dt.bfloat16, tag="fwd")
                if "fwd" in skips:
                    nc.vector.memset(fwd[:, 0:1, 0:1], 0.0)
                else:
                    nc.vector.tensor_tensor(
                        out=fwd[:, :TG, :],
                        in0=_bcast_last(tloc_sb[:, tbase:tbase + TG], P),
                        in1=_bcast_mid(iota_sb[:, :], TG),
                        op=mybir.AluOpType.is_equal)

                # s_tgt expansion into slot space (per tile); hi+lo summed in PSUM
                stgt = pps.tile([P, sched.TGmax, H], dt.float32, tag="stgt")
                if "mm" in skips:
                    nc.vector.memset(stgt[:, 0:1, 0:1], 0.0)
                for t in range(0 if "mm" in skips else TG):
                    w = G * g + wsel[t]
                    nc.tensor.matmul(stgt[:, t, :], ohrev_t[:, t, :],
                                     s_hilo[:, w, 0, :], start=True, stop=False)
                    nc.tensor.matmul(stgt[:, t, :], ohrev_t[:, t, :],
                                     s_hilo[:, w, 1, :], start=False, stop=True)

                if "vec" in skips:
                    res = pool.tile([P, G, HID], dt.float32, tag="res")
                    nc.vector.memset(res[:, 0:1, 0:1], 0.0)
                    o_ap = out[G * g * P:(G * g + G) * P, :]
                    o_ap = o_ap.rearrange("(w p) h -> p w h", p=P)
                    nc.sync.dma_start(out=o_ap, in_=res[:])
                    return
                # group-wide softmax chain
                g1t32 = g1t.bitcast(dt.float32)
                s_sum = pool.tile([P, sched.TGmax, H], dt.float32, tag="s_sum")
                s_act = pool.tile([P, sched.TGmax, H], dt.float32, tag="s_act")
                nc.vector.tensor_tensor(
                    out=s_sum[:, :TG, :], in0=stgt[:, :TG, :],
                    in1=g1t32[:, :TG, cfg.s_src_off:cfg.s_src_off + H],
                    op=mybir.AluOpType.add)
                nc.vector.scalar_tensor_tensor(
                    out=s_act[:, :TG, :], in0=s_sum[:, :TG, :], scalar=cfg.leak,
                    in1=s_sum[:, :TG, :], op0=mybir.AluOpType.mult,
                    op1=mybir.AluOpType.max)
                exp_t = pool.tile([P, sched.TGmax, H], dt.bfloat16, tag="exp_t")
                nc.scalar.activation(out=exp_t[:, :TG, :], in_=s_act[:, :TG, :],
                                     func=mybir.ActivationFunctionType.Exp)
                w_t = pool.tile([P, sched.TGmax, HID], dt.bfloat16, tag="w_t")
                if cfg.fp8:
                    projv = g1t.bitcast(dt.float8e4)[:, :TG, 0:HID]
                else:
                    projv = g1t[:, :TG, 0:HID]
                proj4 = projv.rearrange("p t (h f) -> p t h f", h=H)
                exp4 = _bcast_last(exp_t[:, :TG, :], cfg.F)
                out4 = w_t[:, :TG, :].rearrange("p t (h f) -> p t h f", h=H)
                nc.vector.tensor_tensor(out=out4, in0=proj4, in1=exp4,
                                        op=mybir.AluOpType.mult)

                # aggregation, w-major order
                agg = pps.tile([P, G, HID], dt.float32, tag="agg")
                den = pps.tile([P, G, H], dt.float32, tag="den")
                if "mm" in skips:
                    nc.vector.memset(agg[:, 0:1, 0:1], 0.0)
                    nc.vector.memset(den[:, 0:1, 0:1], 0.0)
                for wi in range(0 if "mm" in skips else G):
                    ts = [t for t in range(TG) if wsel[t] == wi]
                    for i, t in enumerate(ts):
                        st, sp = (i == 0), (i == len(ts) - 1)
                        nc.tensor.matmul(agg[:, wi, :], fwd[:, t, :],
                                         w_t[:, t, :], start=st, stop=sp)
                        nc.tensor.matmul(den[:, wi, :], fwd[:, t, :],
                                         exp_t[:, t, :], start=st, stop=sp)

                # flush both windows
                den_sb = pool.tile([P, G, H, 1], dt.float32, tag="den_sb")
                recip = pool.tile([P, G, H, 1], dt.float32, tag="recip")
                nc.vector.tensor_scalar_add(out=den_sb[:, :, :, 0], in0=den[:],
                                            scalar1=1e-16)
                nc.vector.reciprocal(out=recip[:], in_=den_sb[:])
                z = pool.tile([P, G, HID], dt.float32, tag="z")
                agg4 = agg[:].rearrange("p w (h f) -> p w h f", h=H)
                z4 = z[:].rearrange("p w (h f) -> p w h f", h=H)
                nc.vector.tensor_tensor(out=z4, in0=agg4,
                                        in1=_bcast_last(recip[:, :, :, 0], cfg.F),
                                        op=mybir.AluOpType.mult)
                res = pool.tile([P, G, HID], dt.float32, tag="res")
                nc.vector.scalar_tensor_tensor(
                    out=res[:], in0=z[:], scalar=a_sb[:, 0:1], in1=z[:],
                    op0=mybir.AluOpType.mult, op1=mybir.AluOpType.max)
                o_ap = out[G * g * P:(G * g + G) * P, :]
                o_ap = o_ap.rearrange("(w p) h -> p w h", p=P)
                nc.sync.dma_start(out=o_ap, in_=res[:])

            for rep in range(repeat):
                emit_phase1(rep)
                s_hilo = emit_phase15()
                if rep == 0:
                    emit_cc()
                emit_phase2(rep, s_hilo)
                if rep < repeat - 1:
                    tc.strict_bb_all_engine_barrier()

    nc.compile()
    return nc


def prepare2(cfg: Cfg2, inputs, mode: str = "allgather"):
    X = np.asarray(inputs["in_nodes_features"], np.float32)
    ei = np.asarray(inputs["edge_index"], np.int64)
    W = np.asarray(inputs["W"], np.float32)
    b_lin = np.asarray(inputs["b_lin"], np.float32)
    a_src = np.asarray(inputs["a_src"], np.float32)
    a_tgt = np.asarray(inputs["a_tgt"], np.float32)
    bias = np.asarray(inputs["bias"], np.float32)
    prelu_a = float(np.asarray(inputs["prelu_a"], np.float32))

    assert np.all(b_lin == 0) and np.all(bias == 0), "nonzero bias unsupported"
    assert 0.0 <= prelu_a <= 1.0, "prelu_a outside [0,1] unsupported"

    src, trg = ei[0], ei[1]
    core_of = trg // cfg.shard
    win_of = (trg % cfg.shard) // P
    if mode == "allgather4":
        sk = src // cfg.shard
        sj = src % cfg.shard
        qs = np.asarray(cfg.qstart, np.int64)
        qr = np.asarray(cfg.qrows, np.int64)
        buck_of = np.searchsorted(qs, sj, side="right") - 1
        eidx = sk * qr[buck_of] + (sj - qs[buck_of])
    else:
        buck_of = src // cfg.bucket
        eidx = src - buck_of * cfg.bucket
    counts = np.zeros((cfg.ncores, cfg.NW, cfg.NB), np.int64)
    for k in range(cfg.ncores):
        m = core_of == k
        counts[k] = np.bincount(
            win_of[m] * cfg.NB + buck_of[m],
            minlength=cfg.NW * cfg.NB).reshape(cfg.NW, cfg.NB)
    sched = Sched2(cfg, counts)

    xts = pack_xt2(cfg, X)
    wtp, wap = pack_w2(cfg, W, a_src, a_tgt)
    av = np.full((P, 1), prelu_a, np.float32)
    iota = np.broadcast_to(np.arange(P, dtype=np.float32), (P, P)).astype(BF16)
    iota = np.ascontiguousarray(iota)

    in_maps = []
    for k in range(cfg.ncores):
        g1i_k, tloc_k, ohrev_k = prep_core2(cfg, sched, eidx, buck_of, trg, k)
        in_maps.append({
            "xt": xts[k], "wt": wtp, "wa": wap, "g1i": g1i_k,
            "tlocd": tloc_k, "ohrevd": ohrev_k, "iotad": iota, "avec": av,
        })

    def assemble(core_outs):
        full = np.concatenate(
            [np.asarray(o["out"], np.float32) for o in core_outs], axis=0)
        return full[: cfg.N]

    return sched, in_maps, assemble


_BUILT2 = {}


MODE = "allgather4"


def kernel(**inputs):
    from concourse.bass_utils import run_bass_kernel_spmd

    cfg = Cfg2()
    sched, in_maps, assemble = prepare2(cfg, inputs, mode=MODE)
    key = (tuple(sched.TG), sched.idxcols)
    if key not in _BUILT2:
        _BUILT2[key] = build_nc2(cfg, sched, mode=MODE)
    nc = _BUILT2[key]
    res = run_bass_kernel_spmd(nc, in_maps, core_ids=list(range(cfg.ncores)))
    return assemble(res.results)


# --- back-compat aliases so the existing test.py harness keeps working ---
Cfg = Cfg2


def prepare(cfg, inputs):
    return prepare2(cfg, inputs, mode=MODE)


def build_nc(cfg, sched, phases="full", skip="", repeat=1):
    return build_nc2(cfg, sched, mode=MODE, repeat=repeat, skip=skip)


# revision 29
# speedup vs baseline: 1.2712x; 1.0601x over previous
"""GAT layer (multi-head graph attention) on 8 TRN2 NeuronCores — V2.

Structure (vs V1 baseline):
  phase 1 : projection GEMM sharded 8-way by node rows (98 tiles/core,
            per-core xt input slice); rows packed [proj bf16 | s_src f32 |
            s_tgt f32 | pad] and written to a local DRAM shard.
  CC      : one 8-core AllGather assembles the full 100352-row table
            (pair-shared HBM output).  (mode="pair": each core instead
            computes half the table into pair-shared HBM, tiny AllReduce
            as a fence.)
  phase 2 : destination windows, processed in groups of G=2 windows.
            Per group: 4 dma_gather calls (one per 32768-row src bucket),
            the fwd one-hot is generated on device (tloc vs iota compare),
            the rev one-hot streams from host; s_tgt expansion + softmax
            chain run group-wide; agg/den accumulate in PSUM banks per
            window (w-major matmul order).
"""

import numpy as np
import ml_dtypes

BF16 = ml_dtypes.bfloat16
P = 128


def _ceil(a, b):
    return -(-a // b)


class Cfg2:
    def __init__(self, fp8: bool = False):
        self.fp8 = fp8
        self.N = 100000
        self.E = 800000
        self.HID = 512
        self.HEADS = 8
        self.F = self.HID // self.HEADS
        self.ncores = 8
        self.G = 2
        self.leak = 0.01
        self.bucket = 32768
        self.NTC = 98                      # phase-1 tiles per core
        self.shard = self.NTC * P          # 12544 rows per core
        self.NPAD8 = self.ncores * self.shard   # 100352
        self.NT = self.NPAD8 // P          # 784
        self.NB = _ceil(self.NPAD8, self.bucket)  # 4
        self.NW = self.NTC                 # 98 windows per core
        self.NG = self.NW // self.G        # 49 groups
        # quarter split of each core's shard (tile-aligned) for chunked CC;
        # uneven split keeps gather buckets at [32768,32768,32768,2048] rows
        # (same slot padding as a single CC) while letting CC_q pipeline
        # behind phase-1 quarter q.
        self.qtiles = [32, 32, 32, 2]
        self.qrows = [t * P for t in self.qtiles]           # per-core rows
        self.qstart = np.cumsum([0] + self.qrows)[:-1]      # within-shard row
        self.qtstart = np.cumsum([0] + self.qtiles)[:-1]    # within-shard tile
        self.brows = [self.ncores * r for r in self.qrows]  # bucket rows
        self.KP = min(self.HID, P)
        self.KT = self.HID // self.KP
        self.proj_bytes = self.HID * (1 if fp8 else 2)
        row_bytes = self.proj_bytes + 2 * self.HEADS * 4
        self.row_bytes = _ceil(row_bytes, 256) * 256   # 768 fp8 / 1280 bf16
        self.row_bf = self.row_bytes // 2
        self.row_f32 = self.row_bytes // 4
        self.s_src_off = self.proj_bytes // 4          # f32 units
        self.s_tgt_off = self.s_src_off + self.HEADS


class Sched2:
    """Uniform (core-independent) grouped phase-2 schedule."""

    def __init__(self, cfg: Cfg2, counts: np.ndarray):
        # counts: [ncores, NW, NB]
        maxcnt = counts.max(axis=0)
        self.seg = np.where(maxcnt > 0, _ceil(maxcnt, P) * P, 0).astype(np.int64)
        self.TG = []          # tiles per group
        self.calls = []       # per group: (b, slot_off, nslots, idxcol0)
        self.wsel = []        # per group: window-in-group of each tile
        self.tslot = []       # per group: first slot of each tile
        self.tile_base = []   # first global tile index of each group
        idxcol = 0
        tt = 0
        for g in range(cfg.NG):
            ws = [cfg.G * g + i for i in range(cfg.G)]
            gcalls, gwsel, gtslot = [], [], []
            off = 0
            for b in range(cfg.NB):
                ns = int(sum(self.seg[w, b] for w in ws))
                if ns == 0:
                    continue
                gcalls.append((b, off, ns, idxcol))
                for wi, w in enumerate(ws):
                    s = int(self.seg[w, b])
                    for j in range(s // P):
                        gwsel.append(wi)
                        gtslot.append(off + sum(int(self.seg[w2, b])
                                                for w2 in ws[:wi]) + j * P)
                off += ns
                idxcol += ns // 16
            assert off % P == 0
            self.tile_base.append(tt)
            self.TG.append(off // P)
            tt += off // P
            self.calls.append(gcalls)
            self.wsel.append(gwsel)
            self.tslot.append(gtslot)
        self.TGmax = max(self.TG)
        self.TT = tt
        self.idxcols = idxcol


def prep_core2(cfg: Cfg2, sched: Sched2, eidx, ebuck, trg, k):
    """Per-core streams: g1i idx wrap, tlocP, ohrev."""
    mask = (trg // cfg.shard) == k
    esrc = eidx[mask]
    etrg = trg[mask]
    trel = etrg - k * cfg.shard
    win = trel // P
    buck = ebuck[mask]
    order = np.lexsort((esrc, buck, win))  # src-sorted: ascending gather addrs
    esrc, trel, win, buck = (a[order] for a in (esrc, trel, win, buck))

    key = win * cfg.NB + buck
    starts = np.searchsorted(key, np.arange(cfg.NW * cfg.NB), side="left")
    ends = np.searchsorted(key, np.arange(cfg.NW * cfg.NB), side="right")

    g1i = np.zeros((P, sched.idxcols), np.int16)
    tlocP = np.full((P, sched.TT), 200.0, BF16)

    for g in range(cfg.NG):
        ws = [cfg.G * g + i for i in range(cfg.G)]
        tbase = sched.tile_base[g]
        tof = 0  # tile offset within group
        for (b, slot_off, nslots, idxcol0) in sched.calls[g]:
            idx = np.zeros(nslots, np.int16)
            soff = 0
            for w in ws:
                s = int(sched.seg[w, b])
                if s == 0:
                    continue
                lo, hi = int(starts[w * cfg.NB + b]), int(ends[w * cfg.NB + b])
                cnt = hi - lo
                assert cnt <= s
                idx[soff:soff + cnt] = esrc[lo:hi].astype(np.int16)
                # per-tile tloc / rev one-hot
                tl = (trel[lo:hi] - w * P).astype(np.int64)
                for j in range(s // P):
                    t = tbase + tof
                    s0, s1 = j * P, min((j + 1) * P, cnt)
                    if s1 > s0:
                        rows = np.arange(s0, s1) - s0
                        cols = tl[s0:s1]
                        tlocP[rows, t] = cols.astype(BF16)
                    tof += 1
                soff += s
            blk = idx.reshape(nslots // 16, 16).T
            g1i[:, idxcol0:idxcol0 + nslots // 16] = np.tile(blk, (8, 1))
    return g1i, tlocP


def pack_xt2(cfg: Cfg2, X: np.ndarray):
    """X [N, HID] f32 -> per-core bf16 [KP, NTC, KT, P] slices."""
    Xp = np.zeros((cfg.NPAD8, cfg.HID), np.float32)
    Xp[: cfg.N] = X
    Xb = Xp.astype(BF16)
    v = Xb.reshape(cfg.NT, P, cfg.KT, cfg.KP).transpose(3, 0, 2, 1)  # [KP,NT,KT,P]
    return [np.ascontiguousarray(v[:, k * cfg.NTC:(k + 1) * cfg.NTC])
            for k in range(cfg.ncores)]


def pack_w2(cfg: Cfg2, W, a_src, a_tgt):
    WT = W.T.astype(np.float32)
    wa_s = (W.reshape(cfg.HEADS, cfg.F, cfg.HID)
            * np.asarray(a_src, np.float32).reshape(cfg.HEADS, cfg.F, 1)).sum(1)
    wa_t = (W.reshape(cfg.HEADS, cfg.F, cfg.HID)
            * np.asarray(a_tgt, np.float32).reshape(cfg.HEADS, cfg.F, 1)).sum(1)
    WA = np.concatenate([wa_s.T, wa_t.T], axis=1)
    wt = np.ascontiguousarray(
        WT.astype(BF16).reshape(cfg.KT, cfg.KP, cfg.HID).transpose(1, 0, 2))
    wa = np.ascontiguousarray(
        WA.astype(BF16).reshape(cfg.KT, cfg.KP, 2 * cfg.HEADS).transpose(1, 0, 2))
    return wt, wa


def _bcast_last(ap, n):
    import concourse.bass as bass
    lst = [list(x) for x in ap.ap] + [[0, n]]
    return bass.AP(ap.tensor, ap.offset, lst)


def _bcast_mid(ap, n):
    """Insert a 0-stride dim of size n before the last dim of an AP."""
    import concourse.bass as bass
    lst = [list(x) for x in ap.ap]
    lst = lst[:-1] + [[0, n]] + lst[-1:]
    return bass.AP(ap.tensor, ap.offset, lst)


def build_nc2(cfg: Cfg2, sched: Sched2, mode: str = "allgather",
              repeat: int = 1, cc: str = "on", skip: str = ""):
    skips = set(s for s in skip.split(",") if s)
    import concourse.bacc as bacc
    import concourse.bass as bass
    import concourse.mybir as mybir
    from concourse.tile import TileContext

    dt = mybir.dt
    H, HID, KT, KP, G = cfg.HEADS, cfg.HID, cfg.KT, cfg.KP, cfg.G

    nc = bacc.Bacc("TRN2", target_bir_lowering=False, num_devices=cfg.ncores)

    xt = nc.dram_tensor("xt", [KP, cfg.NTC, KT, P], dt.bfloat16,
                        kind="ExternalInput")
    wt = nc.dram_tensor("wt", [KP, KT, HID], dt.bfloat16, kind="ExternalInput")
    wa = nc.dram_tensor("wa", [KP, KT, 2 * H], dt.bfloat16, kind="ExternalInput")
    g1i = nc.dram_tensor("g1i", [P, sched.idxcols], dt.int16, kind="ExternalInput")
    tlocd = nc.dram_tensor("tlocd", [P, sched.TT], dt.bfloat16,
                           kind="ExternalInput")
    iotad = nc.dram_tensor("iotad", [P, P], dt.bfloat16, kind="ExternalInput")
    identd = nc.dram_tensor("identd", [P, P], dt.bfloat16, kind="ExternalInput")
    avec = nc.dram_tensor("avec", [P, 1], dt.float32, kind="ExternalInput")
    out = nc.dram_tensor("out", [cfg.NW * P, HID], dt.float32,
                         kind="ExternalOutput")

    with TileContext(nc) as tc:
        with tc.tile_pool(name="const", bufs=1) as cpool, \
             tc.tile_pool(name="dram", bufs=1, space="DRAM") as dpool:
            tshard = dpool.tile([cfg.shard, cfg.row_bf], dt.bfloat16)
            if mode == "allgather4":
                tableQ = [dpool.tile([cfg.brows[q], cfg.row_bf], dt.bfloat16,
                                     addr_space="Shared", name=f"tableQ{q}")
                          for q in range(4)]
            else:
                table = dpool.tile([cfg.NPAD8, cfg.row_bf], dt.bfloat16,
                                   addr_space="Shared")
            wt_sb = cpool.tile([KP, KT, HID], dt.bfloat16)
            nc.sync.dma_start(out=wt_sb[:], in_=wt[:, :, :])
            wa_sb = cpool.tile([KP, KT, 2 * H], dt.bfloat16)
            nc.sync.dma_start(out=wa_sb[:], in_=wa[:, :, :])
            a_sb = cpool.tile([P, 1], dt.float32)
            nc.sync.dma_start(out=a_sb[:], in_=avec[:, :])
            g1i_sb = cpool.tile([P, sched.idxcols], dt.int16)
            nc.sync.dma_start(out=g1i_sb[:], in_=g1i[:, :])
            tloc_sb = cpool.tile([P, sched.TT], dt.bfloat16)
            nc.sync.dma_start(out=tloc_sb[:], in_=tlocd[:, :])
            iota_sb = cpool.tile([P, P], dt.bfloat16)
            nc.sync.dma_start(out=iota_sb[:], in_=iotad[:, :])
            id_sb = cpool.tile([P, P], dt.bfloat16)
            nc.sync.dma_start(out=id_sb[:], in_=identd[:, :])

            # ---------------- phase 1: projection table shard ----------------
            def emit_phase1(rep):
                with tc.tile_pool(name=f"p1_{rep}", bufs=3) as xpool, \
                     tc.tile_pool(name=f"p1ps_{rep}", bufs=2, space="PSUM") as psp, \
                     tc.tile_pool(name=f"p1st_{rep}", bufs=3) as stpool:
                    if "p1" in skips:
                        stg = stpool.tile([P, 2, cfg.row_bf], dt.bfloat16,
                                          tag="stg")
                        nc.vector.memset(stg[:], 0.0)
                        nc.sync.dma_start(
                            out=tshard[0:2 * P, :].rearrange(
                                "(two p) r -> p two r", p=P),
                            in_=stg[:])
                        return
                    for j0 in range(0, cfg.NTC, 2):
                        xtile = xpool.tile([KP, 2, KT, P], dt.bfloat16, tag="x")
                        nc.scalar.dma_start(out=xtile[:],
                                            in_=xt[:, j0:j0 + 2, :, :])
                        stg = stpool.tile([P, 2, cfg.row_bf], dt.bfloat16,
                                          tag="stg")
                        stg32 = stg.bitcast(dt.float32)
                        for u in range(2):
                            ps1 = psp.tile([P, HID], dt.float32, tag="ps1")
                            ps2 = psp.tile([P, 2 * H], dt.float32, tag="ps2")
                            for ki in range(KT):
                                nc.tensor.matmul(ps1[:], xtile[:, u, ki, :],
                                                 wt_sb[:, ki, :],
                                                 start=(ki == 0),
                                                 stop=(ki == KT - 1))
                            for ki in range(KT):
                                nc.tensor.matmul(ps2[:], xtile[:, u, ki, :],
                                                 wa_sb[:, ki, :],
                                                 start=(ki == 0),
                                                 stop=(ki == KT - 1))
                            if cfg.fp8:
                                stg8 = stg.bitcast(dt.float8e4)
                                nc.scalar.copy(out=stg8[:, u, 0:HID], in_=ps1[:])
                            else:
                                nc.scalar.copy(out=stg[:, u, 0:HID], in_=ps1[:])
                            nc.vector.tensor_copy(
                                out=stg32[:, u,
                                          cfg.s_src_off:cfg.s_src_off + 2 * H],
                                in_=ps2[:])
                            nc.vector.memset(
                                stg32[:, u, cfg.s_tgt_off + H:cfg.row_f32], 0.0)
                        nc.sync.dma_start(
                            out=tshard[j0 * P:(j0 + 2) * P, :].rearrange(
                                "(two p) r -> p two r", p=P),
                            in_=stg[:])

            # ---------------- phase 1.5: resident s_tgt (hi/lo) --------------
            def emit_phase15():
                ts32 = tshard.bitcast(dt.float32)
                s_ap = ts32[:, cfg.s_tgt_off:cfg.s_tgt_off + H]
                s_ap = s_ap.rearrange("(w p) h -> p w h", p=P)
                s_all = cpool.tile([P, cfg.NW, H], dt.float32)
                nc.sync.dma_start(out=s_all[:], in_=s_ap)
                s_hilo = cpool.tile([P, cfg.NW, 2, H], dt.bfloat16)
                s_hi32 = cpool.tile([P, cfg.NW, H], dt.float32)
                nc.vector.tensor_copy(out=s_hilo[:, :, 0, :], in_=s_all[:])
                nc.vector.tensor_copy(out=s_hi32[:], in_=s_hilo[:, :, 0, :])
                nc.vector.tensor_tensor(out=s_hilo[:, :, 1, :], in0=s_all[:],
                                        in1=s_hi32[:],
                                        op=mybir.AluOpType.subtract)
                return s_hilo

            # ---------------- collective: assemble full table ----------------
            def emit_cc():
                groups = [list(range(cfg.ncores))]
                if mode == "allgather4":
                    for q in range(4):
                        if cc == "off":
                            nc.gpsimd.dma_start(out=tableQ[q][0:P, :],
                                                in_=tshard[0:P, :])
                            continue
                        lo = int(cfg.qstart[q])
                        nc.gpsimd.collective_compute(
                            "AllGather", mybir.AluOpType.bypass,
                            replica_groups=groups,
                            ins=[tshard[lo:lo + cfg.qrows[q], :].opt()],
                            outs=[tableQ[q].opt()],
                        )
                    return
                if cc == "off":
                    # timing-only ablation: satisfy write-before-read
                    nc.gpsimd.dma_start(out=table[0:P, :], in_=tshard[0:P, :])
                    return
                nc.gpsimd.collective_compute(
                    "AllGather",
                    mybir.AluOpType.bypass,
                    replica_groups=groups,
                    ins=[tshard.opt()],
                    outs=[table.opt()],
                )

            # ---------------- phase 2: window groups --------------------------
            def emit_phase2(rep, s_hilo):
                with tc.tile_pool(name=f"p2_{rep}", bufs=2) as pool, \
                     tc.tile_pool(name=f"p2ps_{rep}", bufs=2, space="PSUM") as pps:
                    for g in range(cfg.NG):
                        emit_group(pool, pps, s_hilo, g)

            def emit_group(pool, pps, s_hilo, g):
                TG = sched.TG[g]
                wsel = sched.wsel[g]
                tbase = sched.tile_base[g]
                g1t = pool.tile([P, sched.TGmax, cfg.row_bf], dt.bfloat16,
                                tag="g1t", bufs=3)
                if "gather" in skips:
                    nc.vector.memset(g1t[:, 0:1, 0:1], 0.0)
                for (b, slot_off, nslots, idxcol0) in \
                        ([] if "gather" in skips else sched.calls[g]):
                    if mode == "allgather4":
                        src_ap = tableQ[b][:, :]
                    else:
                        rows = min(cfg.NPAD8,
                                   (b + 1) * cfg.bucket) - b * cfg.bucket
                        src_ap = table[b * cfg.bucket:b * cfg.bucket + rows, :]
                    nc.gpsimd.dma_gather(
                        g1t[:, slot_off // P:(slot_off + nslots) // P, :],
                        src_ap,
                        g1i_sb[:, idxcol0:idxcol0 + nslots // 16],
                        nslots, nslots, cfg.row_bf)
                fwd = pool.tile([P, sched.TGmax, P], dt.bfloat16, tag="fwd")
                if "fwd" in skips:
                    nc.vector.memset(fwd[:, 0:1, 0:1], 0.0)
                else:
                    nc.vector.tensor_tensor(
                        out=fwd[:, :TG, :],
                        in0=_bcast_last(tloc_sb[:, tbase:tbase + TG], P),
                        in1=_bcast_mid(iota_sb[:, :], TG),
                        op=mybir.AluOpType.is_equal)

                # rev one-hot = fwd.T per tile, via PE transpose + ACT copy
                ohrev_t = pool.tile([P, sched.TGmax, P], dt.bfloat16, tag="ohrev")
                if "ohrev" in skips or "mm" in skips:
                    nc.vector.memset(ohrev_t[:, 0:1, 0:1], 0.0)
                else:
                    for t in range(TG):
                        trans = pps.tile([P, P], dt.bfloat16, tag="trans")
                        nc.tensor.transpose(trans[:], fwd[:, t, :], id_sb[:])
                        nc.scalar.copy(out=ohrev_t[:, t, :], in_=trans[:])

                # s_tgt expansion into slot space (per tile); hi+lo summed in
                # PSUM.  den shares the stgt bank (rows TGmax..TGmax+G).
                cm = pps.tile([P, sched.TGmax + G, H], dt.float32, tag="stgt")
                stgt = cm[:, 0:sched.TGmax, :]
                if "mm" in skips:
                    nc.vector.memset(cm[:, 0:1, 0:1], 0.0)
                for t in range(0 if "mm" in skips else TG):
                    w = G * g + wsel[t]
                    nc.tensor.matmul(stgt[:, t, :], ohrev_t[:, t, :],
                                     s_hilo[:, w, 0, :], start=True, stop=False)
                    nc.tensor.matmul(stgt[:, t, :], ohrev_t[:, t, :],
                                     s_hilo[:, w, 1, :], start=False, stop=True)

                if "vec" in skips:
                    res = pool.tile([P, G, HID], dt.float32, tag="res")
                    nc.vector.memset(res[:, 0:1, 0:1], 0.0)
                    o_ap = out[G * g * P:(G * g + G) * P, :]
                    o_ap = o_ap.rearrange("(w p) h -> p w h", p=P)
                    nc.sync.dma_start(out=o_ap, in_=res[:])
                    return
                # group-wide softmax chain
                g1t32 = g1t.bitcast(dt.float32)
                s_sum = pool.tile([P, sched.TGmax, H], dt.float32, tag="s_sum")
                s_act = pool.tile([P, sched.TGmax, H], dt.float32, tag="s_act")
                nc.vector.tensor_tensor(
                    out=s_sum[:, :TG, :], in0=stgt[:, :TG, :],
                    in1=g1t32[:, :TG, cfg.s_src_off:cfg.s_src_off + H],
                    op=mybir.AluOpType.add)
                nc.vector.scalar_tensor_tensor(
                    out=s_act[:, :TG, :], in0=s_sum[:, :TG, :], scalar=cfg.leak,
                    in1=s_sum[:, :TG, :], op0=mybir.AluOpType.mult,
                    op1=mybir.AluOpType.max)
                exp_t = pool.tile([P, sched.TGmax, H], dt.bfloat16, tag="exp_t")
                nc.scalar.activation(out=exp_t[:, :TG, :], in_=s_act[:, :TG, :],
                                     func=mybir.ActivationFunctionType.Exp)
                w_t = pool.tile([P, sched.TGmax, HID], dt.bfloat16, tag="w_t")
                if cfg.fp8:
                    projv = g1t.bitcast(dt.float8e4)[:, :TG, 0:HID]
                else:
                    projv = g1t[:, :TG, 0:HID]
                proj4 = projv.rearrange("p t (h f) -> p t h f", h=H)
                exp4 = _bcast_last(exp_t[:, :TG, :], cfg.F)
                out4 = w_t[:, :TG, :].rearrange("p t (h f) -> p t h f", h=H)
                nc.vector.tensor_tensor(out=out4, in0=proj4, in1=exp4,
                                        op=mybir.AluOpType.mult)

                # aggregation, w-major order; den lives in cm's spare rows
                agg = pps.tile([P, G, HID], dt.float32, tag="agg")
                if "mm" in skips:
                    nc.vector.memset(agg[:, 0:1, 0:1], 0.0)
                for wi in range(0 if "mm" in skips else G):
                    ts = [t for t in range(TG) if wsel[t] == wi]
                    for i, t in enumerate(ts):
                        st, sp = (i == 0), (i == len(ts) - 1)
                        nc.tensor.matmul(agg[:, wi, :], fwd[:, t, :],
                                         w_t[:, t, :], start=st, stop=sp)
                        nc.tensor.matmul(cm[:, sched.TGmax + wi, :], fwd[:, t, :],
                                         exp_t[:, t, :], start=st, stop=sp)

                # flush both windows
                den_sb = pool.tile([P, G, H, 1], dt.float32, tag="den_sb")
                recip = pool.tile([P, G, H, 1], dt.float32, tag="recip")
                nc.vector.tensor_scalar_add(
                    out=den_sb[:, :, :, 0],
                    in0=cm[:, sched.TGmax:sched.TGmax + G, :], scalar1=1e-16)
                nc.vector.reciprocal(out=recip[:], in_=den_sb[:])
                z = pool.tile([P, G, HID], dt.float32, tag="z")
                agg4 = agg[:].rearrange("p w (h f) -> p w h f", h=H)
                z4 = z[:].rearrange("p w (h f) -> p w h f", h=H)
                nc.vector.tensor_tensor(out=z4, in0=agg4,
                                        in1=_bcast_last(recip[:, :, :, 0], cfg.F),
                                        op=mybir.AluOpType.mult)
                res = pool.tile([P, G, HID], dt.float32, tag="res")
                nc.vector.scalar_tensor_tensor(
                    out=res[:], in0=z[:], scalar=a_sb[:, 0:1], in1=z[:],
                    op0=mybir.AluOpType.mult, op1=mybir.AluOpType.max)
                o_ap = out[G * g * P:(G * g + G) * P, :]
                o_ap = o_ap.rearrange("(w p) h -> p w h", p=P)
                nc.sync.dma_start(out=o_ap, in_=res[:])

            for rep in range(repeat):
                emit_phase1(rep)
                s_hilo = emit_phase15()
                if rep == 0:
                    emit_cc()
                emit_phase2(rep, s_hilo)
                if rep < repeat - 1:
                    tc.strict_bb_all_engine_barrier()

    nc.compile()
    return nc


def prepare2(cfg: Cfg2, inputs, mode: str = "allgather"):
    X = np.asarray(inputs["in_nodes_features"], np.float32)
    ei = np.asarray(inputs["edge_index"], np.int64)
    W = np.asarray(inputs["W"], np.float32)
    b_lin = np.asarray(inputs["b_lin"], np.float32)
    a_src = np.asarray(inputs["a_src"], np.float32)
    a_tgt = np.asarray(inputs["a_tgt"], np.float32)
    bias = np.asarray(inputs["bias"], np.float32)
    prelu_a = float(np.asarray(inputs["prelu_a"], np.float32))

    assert np.all(b_lin == 0) and np.all(bias == 0), "nonzero bias unsupported"
    assert 0.0 <= prelu_a <= 1.0, "prelu_a outside [0,1] unsupported"

    src, trg = ei[0], ei[1]
    core_of = trg // cfg.shard
    win_of = (trg % cfg.shard) // P
    if mode == "allgather4":
        sk = src // cfg.shard
        sj = src % cfg.shard
        qs = np.asarray(cfg.qstart, np.int64)
        qr = np.asarray(cfg.qrows, np.int64)
        buck_of = np.searchsorted(qs, sj, side="right") - 1
        eidx = sk * qr[buck_of] + (sj - qs[buck_of])
    else:
        buck_of = src // cfg.bucket
        eidx = src - buck_of * cfg.bucket
    counts = np.zeros((cfg.ncores, cfg.NW, cfg.NB), np.int64)
    for k in range(cfg.ncores):
        m = core_of == k
        counts[k] = np.bincount(
            win_of[m] * cfg.NB + buck_of[m],
            minlength=cfg.NW * cfg.NB).reshape(cfg.NW, cfg.NB)
    sched = Sched2(cfg, counts)

    xts = pack_xt2(cfg, X)
    wtp, wap = pack_w2(cfg, W, a_src, a_tgt)
    av = np.full((P, 1), prelu_a, np.float32)
    iota = np.broadcast_to(np.arange(P, dtype=np.float32), (P, P)).astype(BF16)
    iota = np.ascontiguousarray(iota)
    ident = np.eye(P, dtype=np.float32).astype(BF16)

    in_maps = []
    for k in range(cfg.ncores):
        g1i_k, tloc_k = prep_core2(cfg, sched, eidx, buck_of, trg, k)
        in_maps.append({
            "xt": xts[k], "wt": wtp, "wa": wap, "g1i": g1i_k,
            "tlocd": tloc_k, "iotad": iota, "identd": ident, "avec": av,
        })

    def assemble(core_outs):
        full = np.concatenate(
            [np.asarray(o["out"], np.float32) for o in core_outs], axis=0)
        return full[: cfg.N]

    return sched, in_maps, assemble


_BUILT2 = {}


MODE = "allgather4"


def kernel(**inputs):
    from concourse.bass_utils import run_bass_kernel_spmd

    cfg = Cfg2()
    sched, in_maps, assemble = prepare2(cfg, inputs, mode=MODE)
    key = (tuple(sched.TG), sched.idxcols)
    if key not in _BUILT2:
        _BUILT2[key] = build_nc2(cfg, sched, mode=MODE)
    nc = _BUILT2[key]
    res = run_bass_kernel_spmd(nc, in_maps, core_ids=list(range(cfg.ncores)))
    return assemble(res.results)


# --- back-compat aliases so the existing test.py harness keeps working ---
Cfg = Cfg2


def prepare(cfg, inputs):
    return prepare2(cfg, inputs, mode=MODE)


def build_nc(cfg, sched, phases="full", skip="", repeat=1):
    return build_nc2(cfg, sched, mode=MODE, repeat=repeat, skip=skip)
